# revision 1
# baseline (speedup 1.0000x reference)
"""DepthwiseSeparableAttention Trainium2 kernel (8-core SPMD).

Sharding: core c -> (batch b = c//4, head-group g = c%4, 4 heads each).
Each core computes depthwise-conv + QKV projection for its head slice,
attention for its 4 heads, and a partial output projection; the host sums
the 4 partials per batch and adds the output bias.

All on-device layouts are transposed ([feature, seq]) so the depthwise conv
is a free-dim shift and matmuls contract over partitions.
"""
import os
import sys
for _p in ('/opt/trn_rl_repo', '/root/.axon_site/_ro/trn_rl_repo'):
    if os.path.isdir(_p):
        sys.path.insert(0, _p)
        break

import numpy as np
import ml_dtypes

import concourse.bass as bass
import concourse.mybir as mybir
import concourse.tile as tile
from concourse.vector_clock import ScopedClock

BF16 = mybir.dt.bfloat16
F32 = mybir.dt.float32
AF = mybir.ActivationFunctionType
ALU = mybir.AluOpType

S = 2048          # sequence length
D = 1024          # model dim
DT = 8            # d-tiles of 128
JL = 256          # local head channels (4 heads x 64)
N_CORES = 8

# ---------------------------------------------------------------------------
# walrus in this env allows only ONE sync wait per instruction; split Tile's
# excess waits onto no-fuse NOPs / extra drains.
MAX_WAITS = 1


def _patched_drain_and_barrier(self, tick_clock, wait_clock):
    drain_inst = self.nc.sync.drain()
    wait_clock.add_sem_waits(drain_inst.ins, ScopedClock({None: tick_clock.global_clock}))
    si = drain_inst.ins.sync_info
    if si is not None and len(si.on_wait) > 1:
        waits = list(si.on_wait)
        drain_inst.ins.sync_info = mybir.SyncInfo(on_wait=[waits[0]], on_update=list(si.on_update))
        for w in waits[1:]:
            d2 = self.nc.sync.drain()
            d2.ins.sync_info = mybir.SyncInfo(on_wait=[w], on_update=[])
    self.nc.all_engine_barrier()
    popped = self.nc._tile_sem_poison_stack.pop()
    assert popped is self._sem_poison
    self.nc.clear_and_free_semaphores(list(self.sems.allocated().values()))
    self.nc.all_engine_barrier()


tile.TileContext._drain_and_barrier = _patched_drain_and_barrier


def split_multi_waits(nc):
    n_split = 0
    for f in nc.m.functions:
        for blk in f.blocks:
            il = blk.instructions
            if not any(i.sync_info and len(i.sync_info.on_wait) > MAX_WAITS for i in il):
                continue
            newlist = []
            for inst in il:
                si = inst.sync_info
                if si is not None and len(si.on_wait) > MAX_WAITS:
                    waits = list(si.on_wait)
                    head, tail = waits[:-MAX_WAITS], waits[-MAX_WAITS:]
                    for j, w in enumerate(head):
                        nop = mybir.InstNoOp(
                            name=f"{inst.name}-w{j}",
                            sync_info=mybir.SyncInfo(on_wait=[w], on_update=[]),
                            bass_nofuse=True,
                            engine=inst.engine,
                        )
                        newlist.append(nop)
                        n_split += 1
                    inst.sync_info = mybir.SyncInfo(on_wait=tail, on_update=list(si.on_update))
                newlist.append(inst)
            blk.instructions = newlist
    return n_split


# ---------------------------------------------------------------------------
def build_program(n_rep=1):
    nc = bass.Bass()
    P = {}
    P['xpE'] = nc.declare_dram_parameter("xpE", [128, DT, S + 4], BF16, isOutput=False)
    P['xpO'] = nc.declare_dram_parameter("xpO", [128, DT, S + 4], BF16, isOutput=False)
    for t in ("q", "k", "v"):
        P['w' + t] = nc.declare_dram_parameter("w" + t, [128, DT, JL], BF16, isOutput=False)
        P['tap' + t] = nc.declare_dram_parameter("tap" + t, [128, DT, 3], F32, isOutput=False)
        P['cb' + t] = nc.declare_dram_parameter("cb" + t, [128, DT], F32, isOutput=False)
    P['pbq'] = nc.declare_dram_parameter("pbq", [128, 2], F32, isOutput=False)
    P['pbk'] = nc.declare_dram_parameter("pbk", [128, 2], F32, isOutput=False)
    P['bv2'] = nc.declare_dram_parameter("bv2", [1, JL], BF16, isOutput=False)
    P['wo'] = nc.declare_dram_parameter("wo", [128, 2, D], BF16, isOutput=False)
    P['y'] = nc.declare_dram_parameter("y", [D, S], F32, isOutput=True)
    denom_dram = nc.dram_tensor("denom_scratch", [16, 512], F32)
    rdram2 = nc.dram_tensor("recip_scratch2", [16, 512], F32)

    with tile.TileContext(nc) as tc:
        import contextlib
        with contextlib.ExitStack() as ctx:
            consts = ctx.enter_context(tc.tile_pool(name="consts", bufs=1))
            qkvp = ctx.enter_context(tc.tile_pool(name="qkvp", bufs=1))

            # ---- constants -------------------------------------------------
            w_sb = {}
            tap_sb = {}
            cb_sb = {}
            for t in ("q", "k", "v"):
                w_sb[t] = consts.tile([128, DT, JL], BF16, name="w_" + t)
                nc.sync.dma_start(out=w_sb[t][:], in_=P['w' + t][:])
                tap_sb[t] = consts.tile([128, DT, 3], F32, name="tap_" + t)
                nc.sync.dma_start(out=tap_sb[t][:], in_=P['tap' + t][:])
                cb_sb[t] = consts.tile([128, DT], F32, name="cb_" + t)
                nc.sync.dma_start(out=cb_sb[t][:], in_=P['cb' + t][:])
            pb_sb = {}
            for t in ("q", "k"):
                pb_sb[t] = consts.tile([128, 2], F32, name="pb_" + t)
                nc.sync.dma_start(out=pb_sb[t][:], in_=P['pb' + t][:])
            bv2_sb = consts.tile([1, JL], BF16)
            nc.sync.dma_start(out=bv2_sb[:], in_=P['bv2'][:])
            wo_sb = consts.tile([128, 2, D], BF16)
            nc.sync.dma_start(out=wo_sb[:], in_=P['wo'][:])
            ones_sb = consts.tile([1, 128], BF16)
            nc.vector.memset(ones_sb[:], 1.0)

            # ---- persistent activations -----------------------------------
            for rep in range(n_rep):
              qT = qkvp.tile([128, 2, S], BF16, name="qT")      # [j_in_tile, j_tile, s]
              kT = qkvp.tile([128, 2, S], BF16)
              vx = qkvp.tile([128, 16, 4 * 65], BF16)  # [s_in_tile, s_tile, head*65]
              for h in range(4):
                  nc.vector.memset(vx[:, :, 65 * h + 64: 65 * h + 65], 1.0)

              # ================= phase B: conv + QKV =========================
              with tc.tile_pool(name="bpool", bufs=1) as bpool, \
                   tc.tile_pool(name="convqk", bufs=4) as convqk, \
                   tc.tile_pool(name="convv", bufs=9) as convv, \
                   tc.tile_pool(name="psum_b", bufs=2, space=bass.MemorySpace.PSUM) as psum_b:

                  # Two copies of padded x with different column parity so every
                  # conv tap slice is 4B-aligned (keeps DVE in 2x/4x perf mode).
                  # xpE: x[i] at col 2+i (mid tap, offset 2); xpO: x[i] at col 3+i
                  # (left tap offset 2, right tap offset 4).
                  xpE = bpool.tile([128, DT, S + 4], BF16, name="xpE")
                  xpO = bpool.tile([128, DT, S + 4], BF16, name="xpO")
                  for d in range(DT):
                      nc.sync.dma_start(out=xpE[:, d, :], in_=P['xpE'][:, d, :])
                      nc.sync.dma_start(out=xpO[:, d, :], in_=P['xpO'][:, d, :])

                  def conv_tile(t, d, pool):
                      # conv as TWO partial streams (c1 = mid tap + bias,
                      # c2 = left+right taps); both accumulate in the
                      # projection PSUM, saving one DVE add per tile and
                      # giving the PE weight-alternating back-to-back MMs.
                      sfx = "v" if pool is convv else "qk"
                      cv = pool.tile([128, S], BF16, name="cv_" + sfx)
                      c2 = pool.tile([128, S], BF16, name="c2_" + sfx,
                                     bufs=(2 if pool is convv else None))
                      t0 = pool.tile([128, S], BF16, name="t0_" + sfx, bufs=2)
                      nc.vector.tensor_scalar(
                          out=t0[:], in0=xpO[:, d, 2:S + 2],
                          scalar1=tap_sb[t][:, d, 0:1], scalar2=None, op0=ALU.mult)
                      nc.vector.tensor_scalar(
                          out=c2[:], in0=xpO[:, d, 4:S + 4],
                          scalar1=tap_sb[t][:, d, 2:3], scalar2=None, op0=ALU.mult)
                      nc.vector.tensor_scalar(
                          out=cv[:], in0=xpE[:, d, 2:S + 2],
                          scalar1=tap_sb[t][:, d, 1:2], scalar2=cb_sb[t][:, d:d + 1],
                          op0=ALU.mult, op1=ALU.add)
                      nc.vector.tensor_tensor(out=c2[:], in0=c2[:], in1=t0[:], op=ALU.add)
                      if pool is convv:
                          # v-phase: combine fully (single stream) to fit SBUF
                          nc.vector.tensor_tensor(out=cv[:], in0=cv[:], in1=c2[:], op=ALU.add)
                          return cv, None
                      return cv, c2

                  # q, k projections -> transposed [j, s] layout
                  for t, dst in (("q", qT), ("k", kT)):
                      ps = [psum_b.tile([128, S], F32, name="ps_qk") for _ in range(2)]
                      for d in range(DT):
                          cv, c2 = conv_tile(t, d, convqk)
                          for c in range(4):
                              for s, srcT in enumerate((cv, c2)):
                                  for m in range(2):
                                      nc.tensor.matmul(
                                          ps[m][:, 512 * c: 512 * (c + 1)],
                                          w_sb[t][:, d, 128 * m: 128 * (m + 1)],
                                          srcT[:, 512 * c: 512 * (c + 1)],
                                          start=(d == 0 and s == 0),
                                          stop=(d == DT - 1 and s == 1))
                      for m in range(2):
                          nc.scalar.activation(
                              dst[:, m, :], ps[m][:], AF.Identity,
                              bias=pb_sb[t][:, m: m + 1], scale=1.0)

                  # v projection -> natural [s, j] layout, strided into vx
                  cvv = [conv_tile("v", d, convv) for d in range(DT)]
                  for st in range(16):
                      psv = psum_b.tile([128, S], F32, name="ps_qk")  # share slots
                      for d in range(DT):
                          nc.tensor.matmul(
                              psv[:, 0:JL],
                              cvv[d][0][:, 128 * st: 128 * (st + 1)],
                              w_sb["v"][:, d, :],
                              start=(d == 0), stop=False)
                      nc.tensor.matmul(
                          psv[:, 0:JL], ones_sb[0:1, :], bv2_sb[0:1, :],
                          start=False, stop=True)
                      nc.scalar.copy(
                          vx[:, st, :].rearrange("p (h c) -> p h c", h=4)[:, :, 0:64],
                          psv[:, 0:JL].rearrange("p (h c) -> p h c", h=4))

              # ================= phase C: attention ==========================
              attn_out = qkvp.tile([128, 8, 512], BF16)  # [j_in_pair, pair*4+chunk, qs]
              with tc.tile_pool(name="scores", bufs=3, space=bass.MemorySpace.PSUM) as scorep, \
                   tc.tile_pool(name="attnps", bufs=2, space=bass.MemorySpace.PSUM) as attnp, \
                   tc.tile_pool(name="ptp", bufs=8) as ptp, \
                   tc.tile_pool(name="nrm", bufs=2) as nrmp:
                  for pair in range(2):
                      for chunk in range(4):
                          q0 = 512 * chunk
                          acc = {}
                          for hh in range(2):
                              acc[hh] = attnp.tile([128, 512], F32, name="acc")

                          def emit_scores(ks):
                              sc = scorep.tile([128, 1024], F32, name="sc")
                              for hh in range(2):
                                  r0 = 64 * hh
                                  nc.tensor.matmul(
                                      sc[:, 512 * hh: 512 * (hh + 1)],
                                      kT[r0:r0 + 64, pair, 128 * ks: 128 * (ks + 1)],
                                      qT[r0:r0 + 64, pair, q0: q0 + 512],
                                      start=True, stop=True, tile_position=(r0, 0))
                              p = ptp.tile([128, 1024], BF16, name="pt")
                              nc.scalar.activation(p[:], sc[:], AF.Exp, scale=0.125)
                              return p

                          def emit_attn(ks, p):
                              for hh in range(2):
                                  hl = 2 * pair + hh
                                  nc.tensor.matmul(
                                      acc[hh][0:65, :],
                                      vx[:, ks, 65 * hl: 65 * (hl + 1)],
                                      p[:, 512 * hh: 512 * (hh + 1)],
                                      start=(ks == 0), stop=(ks == 15))

                          # software pipeline: attn lags scores by one ks step
                          prev = None
                          for ks in range(16):
                              p = emit_scores(ks)
                              if prev is not None:
                                  emit_attn(ks - 1, prev)
                              prev = p
                          emit_attn(15, prev)

                          # stash unnormalized output + denominator row
                          idx = 4 * pair + chunk
                          for hh in range(2):
                              den_sb = nrmp.tile([1, 512], F32, name="den_sb")
                              nc.vector.tensor_copy(den_sb[:], acc[hh][64:65, :])
                              nc.sync.dma_start(
                                  out=denom_dram[2 * idx + hh: 2 * idx + hh + 1, :],
                                  in_=den_sb[:])
                              nc.vector.tensor_copy(
                                  attn_out[64 * hh: 64 * (hh + 1), idx, :],
                                  acc[hh][0:64, :])
                      # per-pair batched normalization (overlaps next pair's attention)
                      dn4 = nrmp.tile([8, 512], F32, name="dn4")
                      nc.sync.dma_start(out=dn4[:], in_=denom_dram[8 * pair: 8 * pair + 8, :])
                      rc4 = nrmp.tile([8, 512], F32, name="rc4")
                      nc.vector.reciprocal(rc4[:], dn4[:])
                      nc.sync.dma_start(out=rdram2[8 * pair: 8 * pair + 8, :], in_=rc4[:])
                      for chunk in range(4):
                          idx = 4 * pair + chunk
                          bc = nrmp.tile([128, 512], F32, name="bc")
                          for hh in range(2):
                              rr = rdram2[2 * idx + hh: 2 * idx + hh + 1, :]
                              bc_ap = bass.AP(
                                  tensor=rr.tensor, offset=rr.offset,
                                  ap=[[0, 64]] + list(rr.ap[1:]))
                              nc.gpsimd.dma_start(out=bc[64 * hh: 64 * (hh + 1), :], in_=bc_ap)
                          for hh in range(2):
                              nc.vector.tensor_tensor(
                                  out=attn_out[64 * hh: 64 * (hh + 1), idx, :],
                                  in0=attn_out[64 * hh: 64 * (hh + 1), idx, :],
                                  in1=bc[64 * hh: 64 * (hh + 1), :],
                                  op=ALU.mult)

              # ================= phase D: output projection ==================
              with tc.tile_pool(name="psum_o", bufs=2, space=bass.MemorySpace.PSUM) as psum_o, \
                   tc.tile_pool(name="ypool", bufs=2) as ypool:
                  for m in range(8):
                      ps = psum_o.tile([128, S], F32, name="ps_o")
                      for chunk in range(4):
                          col = 512 * chunk
                          for pair in range(2):
                              nc.tensor.matmul(
                                  ps[:, col: col + 512],
                                  wo_sb[:, pair, 128 * m: 128 * (m + 1)],
                                  attn_out[:, 4 * pair + chunk, :],
                                  start=(pair == 0), stop=(pair == 1))
                      yt = ypool.tile([128, S], F32, name="yt")
                      if m % 2 == 0:
                          nc.vector.tensor_copy(yt[:], ps[:])
                      else:
                          nc.scalar.copy(yt[:], ps[:])
                      nc.sync.dma_start(out=P['y'][128 * m: 128 * (m + 1), :], in_=yt[:])

    split_multi_waits(nc)
    return nc


# ---------------------------------------------------------------------------
def make_in_maps(x, dwq_w, dwq_b, dwk_w, dwk_b, dwv_w, dwv_b,
                 wq, bq, wk, bk, wv, bv, wo, bo):
    bf = ml_dtypes.bfloat16
    in_maps = []
    xp_cache = {}
    for c in range(N_CORES):
        b, g = divmod(c, 4)
        js = slice(JL * g, JL * (g + 1))
        if b not in xp_cache:
            xE = np.zeros((D, S + 4), np.float32)
            xE[:, 2:S + 2] = x[b].T
            xO = np.zeros((D, S + 4), np.float32)
            xO[:, 3:S + 3] = x[b].T
            xp_cache[b] = (
                np.ascontiguousarray(xE.reshape(DT, 128, S + 4).transpose(1, 0, 2)).astype(bf),
                np.ascontiguousarray(xO.reshape(DT, 128, S + 4).transpose(1, 0, 2)).astype(bf))
        m = {'xpE': xp_cache[b][0], 'xpO': xp_cache[b][1]}
        for t, w_, dw_w, dw_b, pb_ in (("q", wq, dwq_w, dwq_b, bq),
                                       ("k", wk, dwk_w, dwk_b, bk),
                                       ("v", wv, dwv_w, dwv_b, bv)):
            m['w' + t] = np.ascontiguousarray(
                w_[js, :].T.reshape(DT, 128, JL).transpose(1, 0, 2)).astype(bf)
            m['tap' + t] = np.ascontiguousarray(
                dw_w.reshape(DT, 128, 3).transpose(1, 0, 2)).astype(np.float32)
            m['cb' + t] = np.ascontiguousarray(dw_b.reshape(DT, 128).T).astype(np.float32)
            if t in ("q", "k"):
                m['pb' + t] = np.ascontiguousarray(pb_[js].reshape(2, 128).T).astype(np.float32)
        m['bv2'] = bv[js].reshape(1, JL).astype(bf)
        m['wo'] = np.ascontiguousarray(
            wo[:, js].T.reshape(2, 128, D).transpose(1, 0, 2)).astype(bf)
        in_maps.append(m)
    return in_maps


def gather_output(results, bo):
    B = 2
    out = np.zeros((B, S, D), np.float32)
    for c in range(N_CORES):
        b = c // 4
        out[b] += results[c]['y'].T
    out += bo
    return out


# ---------------------------------------------------------------------------
_PROGRAM_CACHE = {}


def kernel(x, dwq_w, dwq_b, dwk_w, dwk_b, dwv_w, dwv_b,
           wq, bq, wk, bk, wv, bv, wo, bo):
    """Full-input entry point: shards across 8 NeuronCores internally."""
    from concourse.bass_utils import run_bass_kernel_spmd

    x = np.asarray(x, np.float32)
    args = dict(x=x,
                dwq_w=np.asarray(dwq_w, np.float32), dwq_b=np.asarray(dwq_b, np.float32),
                dwk_w=np.asarray(dwk_w, np.float32), dwk_b=np.asarray(dwk_b, np.float32),
                dwv_w=np.asarray(dwv_w, np.float32), dwv_b=np.asarray(dwv_b, np.float32),
                wq=np.asarray(wq, np.float32), bq=np.asarray(bq, np.float32),
                wk=np.asarray(wk, np.float32), bk=np.asarray(bk, np.float32),
                wv=np.asarray(wv, np.float32), bv=np.asarray(bv, np.float32),
                wo=np.asarray(wo, np.float32), bo=np.asarray(bo, np.float32))
    if 'nc' not in _PROGRAM_CACHE:
        _PROGRAM_CACHE['nc'] = build_program()
    nc = _PROGRAM_CACHE['nc']
    in_maps = make_in_maps(**args)
    res = run_bass_kernel_spmd(nc, in_maps, list(range(N_CORES)))
    return gather_output(res.results, args['bo']).astype(np.float32)

